# revision 5
# baseline (speedup 1.0000x reference)
"""Single-level 2D Haar DWT (analysis) on Trainium2, data-parallel over 8 cores.

Input  x       [16, 64, 256, 256] f32
       weights [1, 1] f32 (w = 1/sqrt(2); the transform scales by w^2)
Output (ll [16, 64, 128, 128], highs [16, 64, 3, 128, 128])

Math (per (n, c) plane, 2x2 polyphase a,b,c,d):
    ll = w2*((a+b)+(c+d)); lh = w2*((c+d)-(a+b));
    hl = w2*((b+d)-(a+c)); hh = w2*((d-c)-(b-a))
Computed as: s = w2*(r0 + r1), t = w2*(r1 - r0) over full rows, then
    ll = s_even + s_odd;  lh = t_even + t_odd
    hl = s_odd - s_even;  hh = t_odd - t_even

Sharding: batch dim 16 -> 2 batches per core, fully local (no collectives).
"""

import os

import numpy as np

B, C, H, W = 16, 64, 256, 256
N_CORES = 8
BL = B // N_CORES  # batches per core
GROUP = 4  # planes (channels) per inner tile group
H2, W2 = H // 2, W // 2

# Stash of the last BassKernelResults (for test harness introspection).
last_results = None


def _build(w2: float, group: int = GROUP, bufs: int = 4):
    import concourse.bacc as bacc
    import concourse.mybir as mybir
    from concourse.tile import TileContext

    f32 = mybir.dt.float32

    nc = bacc.Bacc()
    x = nc.dram_tensor("x", [BL, C, H, W], f32, kind="ExternalInput")
    ll = nc.dram_tensor("ll", [BL, C, H2, W2], f32, kind="ExternalOutput")
    highs = nc.dram_tensor("highs", [BL, C, 3, H2, W2], f32, kind="ExternalOutput")

    n_groups = BL * C // group

    with TileContext(nc) as tc:
        with tc.tile_pool(name="pool", bufs=bufs) as pool:
            for g in range(n_groups):
                n, c0 = divmod(g * group, C)

                # Load `group` full 256x256 planes; partition p holds rows
                # 2p, 2p+1 of each plane (2 KiB contiguous per plane per
                # partition; the whole transfer is contiguous in HBM).
                xin = pool.tile([128, group * 512], f32, tag="xin")
                xv = xin[:].rearrange("p (j t w) -> p j t w", j=group, t=2)
                nc.sync.dma_start(
                    out=xv,
                    in_=x[n, c0 : c0 + group].rearrange("j (p t) w -> p j t w", t=2),
                )
                # xin *= w2 in place (ACT), so downstream ops are plain adds.
                nc.scalar.mul(xin[:], xin[:], w2)
                r0 = xv[:, :, 0, :]
                r1 = xv[:, :, 1, :]

                s_t = pool.tile([128, group * 256], f32, tag="s_t")
                t_t = pool.tile([128, group * 256], f32, tag="t_t")
                sflat = s_t[:].rearrange("p (j w) -> p j w", j=group)
                tflat = t_t[:].rearrange("p (j w) -> p j w", j=group)

                # s = w2*(r0 + r1), t = w2*(r1 - r0)
                nc.vector.tensor_add(sflat, r0, r1)
                nc.vector.tensor_sub(tflat, r1, r0)

                sv = s_t[:].rearrange("p (j w q) -> p j q w", j=group, q=2)
                tv = t_t[:].rearrange("p (j w q) -> p j q w", j=group, q=2)

                ll_t = pool.tile([128, group * 128], f32, tag="ll_t")
                hi_t = pool.tile([128, group * 384], f32, tag="hi_t")
                llv = ll_t[:].rearrange("p (j w) -> p j w", j=group)
                hiv = hi_t[:].rearrange("p (j k w) -> p j k w", j=group, k=3)

                nc.vector.tensor_add(llv, sv[:, :, 0, :], sv[:, :, 1, :])
                nc.vector.tensor_add(hiv[:, :, 0, :], tv[:, :, 0, :], tv[:, :, 1, :])
                nc.vector.tensor_sub(hiv[:, :, 1, :], sv[:, :, 1, :], sv[:, :, 0, :])
                nc.vector.tensor_sub(hiv[:, :, 2, :], tv[:, :, 1, :], tv[:, :, 0, :])

                nc.scalar.dma_start(
                    out=ll[n, c0 : c0 + group].rearrange("j p w -> p j w"),
                    in_=llv,
                )
                nc.scalar.dma_start(
                    out=highs[n, c0 : c0 + group].rearrange("j k p w -> p j k w"),
                    in_=hiv,
                )
    nc.finalize()  # Bacc.finalize runs compile() (reg alloc, wait splitting)
    return nc


def kernel(x, weights):
    global last_results
    from concourse.bass_utils import run_bass_kernel_spmd

    x = np.ascontiguousarray(np.asarray(x, dtype=np.float32))
    wv = np.float32(np.asarray(weights).reshape(-1)[0])
    w2 = float(np.float32(wv * wv))

    nc = _build(w2)
    shards = [
        {"x": np.ascontiguousarray(x[i * BL : (i + 1) * BL])} for i in range(N_CORES)
    ]
    trace = os.environ.get("DWT_TRACE", "0") == "1"
    last_results = run_bass_kernel_spmd(
        nc, shards, core_ids=list(range(N_CORES)), trace=trace
    )
    res = last_results.results
    ll = np.concatenate([r["ll"] for r in res], axis=0)
    highs = np.concatenate([r["highs"] for r in res], axis=0)
    return ll, highs
